# revision 39
# baseline (speedup 1.0000x reference)
"""Single-core fp8 variant: core 0 reduces all 100K nodes alone.

The 8-rank AllReduce costs a fixed ~50µs (runtime pre-exec barrier + op),
and pair-shared-HBM handshakes are exposed to unbounded inter-core launch
skew (0.4-0.9ms observed).  A single core doing everything is slower in
FLOPs/bytes but fully deterministic: 782 fp8 mask-matmul tiles (~44µs PE,
back-to-back at 56ns) overlap a 12.8MB fp8 h load (~41µs DMA), then the
column-form MLP runs locally.  No collective, no cross-core dependence.

  - h is fp8 e4m3 (end-to-end output error ~4e-4 vs the 2e-2 gate) and
    chunk-contiguous in DRAM (17 chunks x 46 tiles, one linear read each).
  - cores 1-7 skip the h load (branch on partition id) and crunch garbage;
    only core 0's output is read.
  - PE warmup matmuls run on uninitialized identb from cycle 0 so the
    p-state clock is ramped before the real loop; MLP weights are packed
    into one [128, 323] f32 DMA with biases as extra contraction rows.
"""
import numpy as np
import ml_dtypes

import concourse.bass as bass
import concourse.mybir as mybir
from concourse.bass_utils import run_bass_kernel_spmd
from concourse.masks import make_identity

NCORES = 8
N = 100000
D = 128
S = 2
GRID_T = 782                # node tiles (100096 padded slots)
NSP = GRID_T * 128
HID = 80
OUT = 2
NWARM = 32                  # PE warmup matmuls (p-state ramp)
NCH = 17                    # h DMA chunks: 46 tiles (5888 cols) each
TPC = GRID_T // NCH
WPK = 3 * HID + HID + OUT + 1   # 323 packed f32 weight columns

_cache = {}


def _build():
    nc = bass.Bass(num_devices=NCORES)
    f32 = mybir.dt.float32
    bf16 = mybir.dt.bfloat16
    f8 = mybir.dt.float8e4

    h_in = nc.dram_tensor("h_bf", [NCH * 128, TPC * 128], f8,
                          kind="ExternalInput")
    m_in = nc.dram_tensor("msk", [128, GRID_T * 16], f8, kind="ExternalInput")
    w_in = nc.dram_tensor("wpk", [128, WPK], f32, kind="ExternalInput")
    e_in = nc.dram_tensor("einit", [128, 1], f32, kind="ExternalInput")
    out_ext = nc.dram_tensor("out", [OUT, 1], f32, kind="ExternalOutput")

    from contextlib import ExitStack
    with ExitStack() as _es:
        _e = _es.enter_context
        h_sb = _e(nc.sbuf_tensor([128, NSP], f8))
        mk_sb = _e(nc.sbuf_tensor([128, GRID_T * 16], f8))
        wp_sb = _e(nc.sbuf_tensor([128, WPK], f32))
        identf = _e(nc.sbuf_tensor([128, 128], f32))
        identb = _e(nc.sbuf_tensor([128, 128], bf16))
        r_sb = _e(nc.sbuf_tensor([S, D], f32))
        rall_sb = _e(nc.sbuf_tensor([D, S], f32))
        x1_sb = _e(nc.sbuf_tensor([128, 1], f32))
        x2_sb = _e(nc.sbuf_tensor([128, 1], f32))
        o_sb = _e(nc.sbuf_tensor([OUT, 1], f32))
        scr_sb = _e(nc.sbuf_tensor([1, 1], f32))
        scr2_sb = _e(nc.sbuf_tensor([1, 1], f32))

        ph0 = _e(nc.psum_tensor([128, 128], f32))
        pr = _e(nc.psum_tensor([S, D], f32))
        ptr = _e(nc.psum_tensor([D, S], f32))
        px1 = _e(nc.psum_tensor([HID, 1], f32))
        px2 = _e(nc.psum_tensor([HID, 1], f32))
        po = _e(nc.psum_tensor([OUT, 1], f32))

        s_id = _e(nc.semaphore("s_id"))
        s_mk = _e(nc.semaphore("s_mk"))
        s_w = _e(nc.semaphore("s_w"))
        s_hc = [_e(nc.semaphore(f"s_h{i}")) for i in range(NCH)]
        s_rr = _e(nc.semaphore("s_rr"))
        s_rs = _e(nc.semaphore("s_rs"))
        s_tr = _e(nc.semaphore("s_tr"))
        s_ra = _e(nc.semaphore("s_ra"))
        s_x1 = _e(nc.semaphore("s_x1"))
        s_a1 = _e(nc.semaphore("s_a1"))
        s_x2 = _e(nc.semaphore("s_x2"))
        s_a2 = _e(nc.semaphore("s_a2"))
        s_x3 = _e(nc.semaphore("s_x3"))
        s_ov = _e(nc.semaphore("s_ov"))
        block = _e(nc.Block())

        @block.sync
        def _(sy):
            # mask + first h chunk load on every core (cheap), so the
            # r-loop can start without waiting on the branch resolution
            sy.dma_start(out=mk_sb[:], in_=m_in[:]).then_inc(s_mk, 16)
            sy.dma_start(out=h_sb[:, 0:TPC * 128],
                         in_=h_in[0:128, :]).then_inc(s_hc[0], 16)
            # only core 0 loads the remaining node features; the rest just
            # bump the semaphores (their r is garbage and unread)
            with sy.register("sid") as sid:
                sy.reg_load(sid, nc.partition_id_tensor[0:1, 0:1])
                with sy.If_lt(sid, 1):
                    for ci in range(1, NCH):
                        sy.dma_start(
                            out=h_sb[:, ci * TPC * 128:(ci + 1) * TPC * 128],
                            in_=h_in[ci * 128:(ci + 1) * 128, :]
                        ).then_inc(s_hc[ci], 16)
                with sy.Else():
                    for ci in range(1, NCH):
                        sy.nop().then_inc(s_hc[ci], 16)
            sy.dma_start(out=wp_sb[:], in_=w_in[:]).then_inc(s_w, 16)
            sy.dma_start(out=x1_sb[:], in_=e_in[:]).then_inc(s_w, 16)
            sy.dma_start(out=x2_sb[:], in_=e_in[:]).then_inc(s_w, 16)
            sy.wait_ge(s_ov, 1)
            sy.dma_start(out=out_ext[:], in_=o_sb[:]).then_inc(s_w, 16)

        @block.gpsimd
        def _(g):
            make_identity(nc, identb[:])
            make_identity(nc, identf[:])
            nc.gpsimd.memset(scr_sb[:], 0.0).then_inc(s_id, 1)

        @block.vector
        def _(v):
            v.wait_ge(s_rr, 1)
            v.tensor_copy(out=r_sb[:], in_=pr[:]).then_inc(s_rs, 1)
            v.wait_ge(s_tr, 1)
            v.tensor_copy(out=rall_sb[:], in_=ptr[:]).then_inc(s_ra, 1)
            v.wait_ge(s_x3, 1)
            v.tensor_copy(out=o_sb[:], in_=po[:]).then_inc(s_ov, 1)

        @block.scalar
        def _(sc):
            # dummy activation: preload the ACT table off the critical path
            sc.wait_ge(s_id, 1)
            nc.scalar.activation(out=scr2_sb[:], in_=scr_sb[:],
                                 func=mybir.ActivationFunctionType.Lrelu,
                                 alpha=0.01)
            sc.wait_ge(s_w, 48)   # x1/x2 init DMAs must land before ACT writes
            sc.wait_ge(s_x1, 1)
            nc.scalar.activation(out=x1_sb[0:HID, 0:1], in_=px1[:],
                                 func=mybir.ActivationFunctionType.Lrelu,
                                 alpha=0.01).then_inc(s_a1, 1)
            sc.wait_ge(s_x2, 1)
            nc.scalar.activation(out=x2_sb[0:HID, 0:1], in_=px2[:],
                                 func=mybir.ActivationFunctionType.Lrelu,
                                 alpha=0.01).then_inc(s_a2, 1)

        @block.tensor
        def _(t):
            # warmup on uninitialized identb: p-state ramp only, output unread
            for k in range(NWARM):
                nc.tensor.matmul(out=ph0[:], lhsT=identb[:], rhs=identb[:],
                                 start=True, stop=True)
            t.wait_ge(s_mk, 16)
            mm = None
            ndt = GRID_T // 2
            for ci in range(NCH):
                t.wait_ge(s_hc[ci], 16)
                for dt_ in range(ci * TPC // 2, (ci + 1) * TPC // 2):
                    # fp8 DoubleRow: two 128-node k-tiles per instruction.
                    # k-tile dim: count 2, stride %16==0 (mask cols 16 apart,
                    # h tiles 128 apart).
                    mb = mk_sb[:, 32 * dt_:32 * dt_ + 32]
                    lT = bass.AP(mb.tensor, mb.offset,
                                 [list(mb.ap[0]), [16, 2], [1, S]])
                    hb = h_sb[:, 256 * dt_:256 * (dt_ + 1)]
                    rh = bass.AP(hb.tensor, hb.offset,
                                 [list(hb.ap[0]), [128, 2], [1, 128]])
                    mm = nc.tensor.matmul(
                        out=pr[:], lhsT=lT, rhs=rh,
                        perf_mode=mybir.MatmulPerfMode.DoubleRow,
                        start=(dt_ == 0), stop=(dt_ == ndt - 1))
            mm.then_inc(s_rr, 1)
            # transpose r [2,128] -> [128,2]
            t.wait_ge(s_id, 1)
            t.wait_ge(s_rs, 1)
            nc.tensor.transpose(out=ptr[:], in_=r_sb[:],
                                identity=identf[:S, :S]).then_inc(s_tr, 1)
            # MLP, column form
            t.wait_ge(s_w, 48)
            t.wait_ge(s_ra, 1)
            nc.tensor.matmul(out=px1[:], lhsT=wp_sb[:, 0:HID],
                             rhs=rall_sb[:, 0:1], start=True, stop=False)
            nc.tensor.matmul(out=px1[:], lhsT=wp_sb[:, HID:2 * HID],
                             rhs=rall_sb[:, 1:2], start=False, stop=False)
            nc.tensor.matmul(out=px1[:], lhsT=wp_sb[:, 2 * HID:3 * HID],
                             rhs=wp_sb[:, WPK - 1:WPK], start=False,
                             stop=True).then_inc(s_x1, 1)
            t.wait_ge(s_a1, 1)
            nc.tensor.matmul(out=px2[:], lhsT=wp_sb[:, 3 * HID:4 * HID],
                             rhs=x1_sb[:], start=True, stop=True
                             ).then_inc(s_x2, 1)
            t.wait_ge(s_a2, 1)
            nc.tensor.matmul(out=po[:], lhsT=wp_sb[:, 4 * HID:4 * HID + OUT],
                             rhs=x2_sb[:], start=True, stop=True
                             ).then_inc(s_x3, 1)

    return nc


def _shard(inputs):
    nodes = np.ascontiguousarray(np.asarray(inputs["nodes"], dtype=np.float32))
    edges = np.asarray(inputs["edges"])
    dst = np.asarray(edges[:, :, 1], dtype=np.int64)

    W = np.asarray(inputs["W"], np.float32)
    f1 = np.asarray(inputs["fc1_w"], np.float32)
    f2 = np.asarray(inputs["fc2_w"], np.float32)
    f3 = np.asarray(inputs["fc3_w"], np.float32)
    wpk = np.zeros((128, WPK), np.float32)
    for s in range(S):
        wpk[:, s * HID:(s + 1) * HID] = (W[s] @ f1[:, :D].T) / N
    wpk[0, 2 * HID:3 * HID] = f1[:, D]
    wpk[1, 2 * HID:3 * HID] = np.asarray(inputs["fc1_b"], np.float32)
    wpk[:HID, 3 * HID:4 * HID] = f2.T
    wpk[HID, 3 * HID:4 * HID] = np.asarray(inputs["fc2_b"], np.float32)
    wpk[:HID, 4 * HID:4 * HID + OUT] = f3.T
    wpk[HID, 4 * HID:4 * HID + OUT] = np.asarray(inputs["fc3_b"], np.float32)
    wpk[0, WPK - 1] = np.asarray(inputs["problem_type"], np.float32)[0, 0]
    wpk[1, WPK - 1] = 1.0
    einit = np.zeros((128, 1), np.float32)
    einit[HID, 0] = 1.0

    hb = np.zeros((NSP, D), np.float32)
    hb[:N] = nodes
    h_old = hb.reshape(GRID_T, 128, D).transpose(1, 0, 2).reshape(128, NSP)
    h_bf = np.ascontiguousarray(
        h_old.reshape(128, NCH, TPC * 128).transpose(1, 0, 2)
        .reshape(NCH * 128, TPC * 128).astype(ml_dtypes.float8_e4m3))
    msk = np.zeros((128, GRID_T * 16), np.float32)
    for s in range(S):
        pres = np.zeros((NSP,), np.float32)
        pres[dst[s]] = 1.0
        pt = pres.reshape(GRID_T, 128).T   # [128, GRID_T]
        msk[:, s::32] = pt[:, 0::2]        # even tile of each pair
        msk[:, 16 + s::32] = pt[:, 1::2]   # odd tile, 16 cols apart
    msk = np.ascontiguousarray(msk.astype(ml_dtypes.float8_e4m3))

    zero_h = np.zeros((NCH * 128, TPC * 128), ml_dtypes.float8_e4m3)
    zero_m = np.zeros((128, GRID_T * 16), ml_dtypes.float8_e4m3)

    per_core = []
    for c in range(NCORES):
        per_core.append({
            "h_bf": h_bf if c == 0 else zero_h,
            "msk": msk if c == 0 else zero_m,
            "wpk": wpk,
            "einit": einit,
        })
    return per_core


def kernel(trace=False, **inputs) -> np.ndarray:
    if "nc" not in _cache:
        _cache["nc"] = _build()
    nc = _cache["nc"]
    in_maps = _shard(inputs)
    res = run_bass_kernel_spmd(nc, in_maps, core_ids=list(range(NCORES)),
                               trace=trace)
    _cache["last_result"] = res
    return np.asarray(res.results[0]["out"], np.float32).reshape(1, OUT)
